# revision 27
# baseline (speedup 1.0000x reference)
"""Trainium2 Bass kernel for a single attention head.

reference computation (fp32):
    q = query @ Wq + bq ; k = key @ Wk + bk ; v = value @ Wv + bv
    out = softmax((q @ k^T) / 8) @ v

Sharding: 8 cores, core c -> (batch b = c//2, query-half h = c%2).
Each core computes attention for its 2048 query rows against the full 4096
keys/values of its batch.

Design (bf16 pipeline, host-transposed inputs; rel err ~7e-3 vs 2e-2 gate):
  - host supplies X^T slices in bf16 ([512, rows], c-major) so activations
    DMA straight into SBUF in the projection-ready layout: no PE transposes
    of X, no PSUM->SBUF staging copies, and half the HBM traffic of fp32.
    All weights are packed into one bf16 DMA, biases into one fp32 DMA.
  - projections on PE (bf16): lhsT = W [c-chunk, d], rhs = X^T chunk;
    bias folded into the mandatory PSUM->SBUF copy (DVE tensor_scalar_add).
    Qp^T [64, 2048] is duplicated to partitions 64:128 (sync-queue SBUF
    DMA); Kp^T is stored dual-half (even j-chunks on partitions 0:64, odd
    on 64:128) by issuing separate even/odd matmuls whose outputs land at
    partition offsets 0/64 via tile_position - no partition-shift DMA.
  - V is projected then PE-transposed to natural [rows, 66]; col 64 is
    all-ones (host pads Wv/bv) so the PV matmul also produces the softmax
    denominator; col 65 is zero padding.
  - scores^T tiles: lhsT = Kp^T[half, j-chunk] [64,128], rhs = Qp^T
    [64, 1024-i-slice] -> S^T [128 j, i] in PSUM; the two K=64 matmuls of a
    j-chunk pair occupy PE row-groups 0:64 / 64:128 (tile_position row
    tiling); exp fused with the 1/8 scale on ScalarE, output bf16 (no
    max-subtraction: |scores/8| <= ~3 so fp32 exp is safe).
  - PV: lhsT = v[j-chunk] [128, 66] bf16, rhs = P^T [128, i] bf16,
    accumulated over j in PSUM -> out^T [66, i] (row 64 = denominator).
  - epilogue: PE-transpose out^T, reciprocal on DVE, scale via Copy
    activation; the final epilogue runs its copies/scales on the (by then
    idle) ScalarE so the DVE chain is off the tail's critical path.
  - scheduling: phase A (prework + i-half 0) is PE-bound while phase B
    (i-half 1) is ACT-bound, so the PV of several ih0 pairs is deferred
    into phase B: their exp outputs stay live in the pt pool and drain
    while ih1 scores/exp keep ACT saturated through the transition. The
    first pair's exps are split per 512-column slice so ACT starts as soon
    as the first q block lands.
"""

import sys

if "/opt/trn_rl_repo" not in sys.path:
    sys.path.insert(0, "/opt/trn_rl_repo")

from contextlib import ExitStack

import numpy as np
import ml_dtypes

import concourse.bass as bass
import concourse.tile as tile
from concourse import bacc, mybir
from concourse.bass_utils import run_bass_kernel_spmd
from concourse.masks import make_identity

F32 = mybir.dt.float32
BF16 = mybir.dt.bfloat16
NP_BF16 = ml_dtypes.bfloat16

B, S, C, D = 4, 4096, 512, 64
D2 = D + 2          # v padded with [ones, zeros] cols
WW = 4 * (D + D + D2)  # packed weight row: 776 bf16 per partition
N_CORES = 8
SQ = S // 2          # query rows per core
NJ = S // 128        # 32 key chunks of 128 rows
NP_ = NJ // 2        # 16 chunk pairs
IH = SQ // 2         # 1024: i-half processed per PSUM residency
EXP = mybir.ActivationFunctionType.Exp
COPY = mybir.ActivationFunctionType.Copy

# ih0 pairs whose PV is deferred into phase B (spread so phase A's per-g
# PE load stays balanced against ACT)
DEFER = frozenset((3, 5, 7, 9, 11, 13, 15))

_CACHE = {}


def _emit(nc, tc, aps):
    qT_d, kT_d, vT_d, w_d, b_d, out_d = aps

    ctx = ExitStack()
    const = ctx.enter_context(tc.tile_pool(name="const", bufs=1))
    persist = ctx.enter_context(tc.tile_pool(name="persist", bufs=1))
    stage_p = ctx.enter_context(tc.tile_pool(name="stage", bufs=4))
    vt_p = ctx.enter_context(tc.tile_pool(name="vt", bufs=2))
    pt_p = ctx.enter_context(tc.tile_pool(name="pt", bufs=34))
    ep_p = ctx.enter_context(tc.tile_pool(name="ep", bufs=2))
    small_p = ctx.enter_context(tc.tile_pool(name="small", bufs=4))
    out_p = ctx.enter_context(tc.tile_pool(name="outp", bufs=2))
    # PSUM budget (8 banks): scratch 2x1 + st 2x2 + po 1x2 = 8
    pp_ps = ctx.enter_context(tc.tile_pool(name="ppps", bufs=2, space="PSUM"))
    st_ps = ctx.enter_context(tc.tile_pool(name="stps", bufs=2, space="PSUM"))
    po_ps = ctx.enter_context(tc.tile_pool(name="pops", bufs=1, space="PSUM"))

    # one packed DMA for all weights, one for all biases
    w_all = const.tile([128, 4, D + D + D2], BF16)
    nc.sync.dma_start(w_all[:].rearrange("p cc d -> p (cc d)"), w_d[:])
    wq_sb = w_all[:, :, 0:D]
    wk_sb = w_all[:, :, D : 2 * D]
    wvp_sb = w_all[:, :, 2 * D : 2 * D + D2]
    b_all = const.tile([128, 3], F32)
    nc.sync.dma_start(b_all[:], b_d[:])
    bq_sb = b_all[:D, 0:1]
    bk2_sb = b_all[:, 1:2]
    bvp_sb = b_all[:D2, 2:3]

    ident = const.tile([128, 128], BF16)

    qpt = persist.tile([128, SQ], BF16)      # Qp^T duplicated on both halves
    kpt = persist.tile([128, S // 2], BF16)  # Kp^T dual-half (even|odd chunks)
    v_sb = persist.tile([128, NJ, D2], BF16)  # v natural + ones col

    def load_block(x_d, g):
        """DMA 512 c x 512 rows of a host-transposed activation into SBUF.
        SWDGE (gpsimd): keeps engine HWDGE queues free; Pool is idle anyway."""
        stg = stage_p.tile([128, 4, 512], BF16, tag="stage")
        nc.gpsimd.dma_start(
            stg[:],
            x_d[:, g * 512 : (g + 1) * 512].rearrange("(cc p) r -> p cc r", p=128),
        )
        return stg

    def proj_q(g):
        stg = load_block(qT_d, g)
        pp = pp_ps.tile([128, 512], F32, tag="pp")
        for cc in range(4):
            nc.tensor.matmul(
                pp[:D, :], wq_sb[:, cc, :], stg[:, cc, :],
                start=(cc == 0), stop=(cc == 3),
            )
        sl = slice(g * 512, (g + 1) * 512)
        nc.vector.tensor_scalar_add(qpt[:D, sl], pp[:D, :], bq_sb)
        # duplicate to partitions 64:128 via the sync HWDGE queue (idle
        # mid-kernel, so its dep-wait blocks nothing)
        nc.sync.dma_start(qpt[D:, sl], qpt[:D, sl])

    def proj_k(g):
        # block g covers j-chunks 4g..4g+3; even chunks project to output
        # partitions 0:64, odd to 64:128 (tile_position col offset), so the
        # bias-add writes kpt's dual-half layout directly.
        stg = load_block(kT_d, g)
        pp = pp_ps.tile([128, 512], F32, tag="pp")
        for half in range(2):
            for cc in range(4):
                rhs = stg[:, cc, :].rearrange("p (c n) -> p c n", n=128)[:, half::2, :]
                nc.tensor.matmul(
                    pp[half * D : (half + 1) * D, :256],
                    wk_sb[:, cc, :],
                    rhs,
                    start=(cc == 0), stop=(cc == 3),
                    tile_position=(0, half * D),
                )
        sl = slice(g * 256, (g + 1) * 256)
        nc.vector.tensor_scalar_add(kpt[:D, sl], pp[:D, :256], bk2_sb[:D, :])
        nc.vector.tensor_scalar_add(kpt[D:, sl], pp[D:, :256], bk2_sb[D:, :])

    def proj_v(g):
        stg = load_block(vT_d, g)
        pp = pp_ps.tile([128, 512], F32, tag="pp")
        for cc in range(4):
            nc.tensor.matmul(
                pp[:D2, :], wvp_sb[:, cc, :], stg[:, cc, :],
                start=(cc == 0), stop=(cc == 3),
            )
        vt = vt_p.tile([D2, 512], BF16, tag="vt")
        nc.vector.tensor_scalar_add(vt[:], pp[:D2, :], bvp_sb)
        for r in range(4):
            vnp = pp_ps.tile([128, D2], BF16, tag="pp")
            nc.tensor.transpose(
                vnp[:], vt[:, r * 128 : (r + 1) * 128], ident[:D2, :D2]
            )
            nc.vector.tensor_copy(v_sb[:, g * 4 + r, :], vnp[:])

    def scores_exp(p, ih, nw=IH):
        # chunk pair p = chunks (2p, 2p+1): even on kpt rows 0:64, odd 64:128.
        # nw < 512 splits the matmul/exp into narrower column slices so the
        # first pair can start before the full q width has landed.
        sts = []
        for half in range(2):
            st = st_ps.tile([128, IH], F32, tag="st")
            pt = pt_p.tile([128, IH], BF16, tag="pt")
            for n in range(IH // 512):   # matmul out must fit one PSUM bank
                nc.tensor.matmul(
                    st[:, n * 512 : (n + 1) * 512],
                    kpt[half * D : (half + 1) * D, p * 128 : (p + 1) * 128],
                    qpt[half * D : (half + 1) * D,
                        ih * IH + n * 512 : ih * IH + (n + 1) * 512],
                    tile_position=(half * D, 0),
                )
                for m in range(512 // nw):
                    if nw < IH:
                        lo = n * 512 + m * nw
                        nc.scalar.activation(
                            pt[:, lo : lo + nw], st[:, lo : lo + nw],
                            EXP, scale=0.125,
                        )
            if nw == IH:
                nc.scalar.activation(pt[:], st[:], EXP, scale=0.125)
            sts.append(pt)
        return sts

    def pv(p, po, sts, first, last):
        # n-major on the last pair so po's low columns finish first and the
        # epilogue's first chunks unblock while the rest still accumulates
        order = [(h, n) for n in range(IH // 512) for h in range(2)] if last \
            else [(h, n) for h in range(2) for n in range(IH // 512)]
        for half, n in order:
            nc.tensor.matmul(
                po[:, n * 512 : (n + 1) * 512],
                v_sb[:, 2 * p + half, :],
                sts[half][:, n * 512 : (n + 1) * 512],
                start=(first and half == 0), stop=(last and half == 1),
            )

    def epilogue(ih, po, on_act):
        # chunked so po frees early and the final DMA launches with minimal
        # serial tail. on_act (the final epilogue, tail-latency critical):
        # chunks 0,1 run their copy/scale chain on DVE while 2,3 run on the
        # by-then-idle ACT, halving the serial tail.
        osb = out_p.tile([128, IH // 128, D], F32, tag="osb")
        for c in range(4):
            act_c = on_act and c >= 2
            ot = ep_p.tile([D2, 256], BF16, tag="ot")
            if act_c:
                nc.scalar.activation(ot[:], po[:, c * 256 : (c + 1) * 256], COPY)
            else:
                nc.vector.tensor_copy(ot[:], po[:, c * 256 : (c + 1) * 256])
            for tt in range(2):
                t = c * 2 + tt
                onat = pp_ps.tile([128, D2], BF16, tag="pp")
                nc.tensor.transpose(
                    onat[:], ot[:, tt * 128 : (tt + 1) * 128], ident[:D2, :D2]
                )
                rs = small_p.tile([128, 1], F32, tag="rs")
                nc.vector.reciprocal(rs[:], onat[:, D : D + 1])
                if act_c:
                    nc.scalar.activation(osb[:, t, :], onat[:, :D], COPY, scale=rs[:])
                else:
                    nc.vector.tensor_scalar_mul(osb[:, t, :], onat[:, :D], rs[:])
            if c % 2 == 1:
                nc.sync.dma_start(
                    out_d[ih * IH + (c - 1) * 256 : ih * IH + (c + 1) * 256, :]
                    .rearrange("(t p) d -> p t d", p=128),
                    osb[:, c * 2 - 2 : c * 2 + 2, :],
                )

    # emission order = per-engine program order; sequence so the first exp
    # lands as early as possible and ACT never waits on deferrable PE work
    po0 = po_ps.tile([D2, IH], F32, tag="po")
    deferred = []
    proj_k(0)
    proj_q(0)
    proj_q(1)
    ident32 = small_p.tile([128, 128], F32, tag="id32")
    make_identity(nc, ident32[:])           # after the head loads: Pool must
    nc.vector.tensor_copy(ident[:], ident32[:])  # not delay k0/q0 desc-gen
    for g in range(8):
        sts0 = scores_exp(2 * g, 0, nw=(512 if g == 0 else IH))
        if g < 7:
            proj_k(g + 1)       # keep the k pipeline one block ahead
        proj_v(g)
        if 2 * g in DEFER:
            deferred.append((2 * g, sts0))
        else:
            pv(2 * g, po0, sts0, first=(g == 0), last=False)
        sts1 = scores_exp(2 * g + 1, 0)
        if g in (5, 6):
            proj_q(g - 3)       # q cols 1024:2048, needed only for i-half 1
        if 2 * g + 1 in DEFER:
            deferred.append((2 * g + 1, sts1))
        else:
            pv(2 * g + 1, po0, sts1, first=False, last=False)

    # drain deferred ih0 PVs while ih1 scores/exp keep ACT saturated
    exped = {}
    for i, (p0, sts) in enumerate(deferred):
        exped[i] = scores_exp(i, 1)
        pv(p0, po0, sts, first=False, last=(i == len(deferred) - 1))
    epilogue(0, po0, on_act=False)
    # po1 phase: lookahead pipeline — scores/exp stay ~AH pairs ahead of PV
    AH = len(deferred)
    po1 = po_ps.tile([D2, IH], F32, tag="po")
    for p in range(NP_):
        if p + AH < NP_:
            exped[p + AH] = scores_exp(p + AH, 1)
        pv(p, po1, exped.pop(p), first=(p == 0), last=(p == NP_ - 1))
    epilogue(1, po1, on_act=True)
    ctx.close()


def _build(reps=1):
    nc = bacc.Bacc("TRN2", target_bir_lowering=False, debug=False, num_devices=N_CORES)
    aps = (
        nc.dram_tensor("qT", [C, SQ], BF16, kind="ExternalInput").ap(),
        nc.dram_tensor("kT", [C, S], BF16, kind="ExternalInput").ap(),
        nc.dram_tensor("vT", [C, S], BF16, kind="ExternalInput").ap(),
        nc.dram_tensor("w", [128, WW], BF16, kind="ExternalInput").ap(),
        nc.dram_tensor("b", [128, 3], F32, kind="ExternalInput").ap(),
        nc.dram_tensor("out", [SQ, D], F32, kind="ExternalOutput").ap(),
    )
    with tile.TileContext(nc) as tc:
        for _ in range(reps):
            _emit(nc, tc, aps)
    nc.compile()
    return nc


def get_nc():
    if "nc" not in _CACHE:
        _CACHE["nc"] = _build()
    return _CACHE["nc"]


def make_in_maps(query, key_, value, Wq, bq, Wk, bk, Wv, bv):
    query, key_, value, Wq, bq, Wk, bk, Wv, bv = (
        np.asarray(a, dtype=np.float32)
        for a in (query, key_, value, Wq, bq, Wk, bk, Wv, bv)
    )
    wvp = np.concatenate([Wv, np.zeros((C, 2), np.float32)], axis=1)
    # packed weights: per partition p, 4 c-chunks of [wq | wk | wvp] rows
    wcat = np.concatenate([Wq, Wk, wvp], axis=1)          # [512, 194]
    w = np.ascontiguousarray(
        wcat.reshape(4, 128, D + D + D2).transpose(1, 0, 2).reshape(128, WW)
        .astype(NP_BF16)
    )
    bvp = np.concatenate([bv, [1.0, 0.0], np.zeros(128 - D2, np.float32)])
    b = np.ascontiguousarray(
        np.stack(
            [
                np.concatenate([bq, np.zeros(64, np.float32)]),
                np.concatenate([bk, bk]),
                bvp,
            ],
            axis=1,
        ).astype(np.float32)
    )
    kT = [np.ascontiguousarray(key_[b_].T.astype(NP_BF16)) for b_ in range(B)]
    vT = [np.ascontiguousarray(value[b_].T.astype(NP_BF16)) for b_ in range(B)]
    in_maps = []
    for c in range(N_CORES):
        b_, h = divmod(c, 2)
        in_maps.append(
            {
                "qT": np.ascontiguousarray(
                    query[b_, h * SQ : (h + 1) * SQ, :].T.astype(NP_BF16)
                ),
                "kT": kT[b_],
                "vT": vT[b_],
                "w": w,
                "b": b,
            }
        )
    return in_maps


def assemble(results):
    out = np.empty((B, S, D), np.float32)
    for c in range(N_CORES):
        b_, h = divmod(c, 2)
        out[b_, h * SQ : (h + 1) * SQ, :] = results[c]["out"]
    return out


def kernel(query=None, key_=None, value=None, Wq=None, bq=None, Wk=None,
           bk=None, Wv=None, bv=None, key=None, **_):
    if key_ is None:
        key_ = key          # spec names this input "key"; reference uses "key_"
    nc = get_nc()
    in_maps = make_in_maps(query, key_, value, Wq, bq, Wk, bk, Wv, bv)
    res = run_bass_kernel_spmd(nc, in_maps, list(range(N_CORES)))
    return assemble(res.results)


# revision 30
# speedup vs baseline: 1.8402x; 1.8402x over previous
"""Trainium2 Bass kernel for a single attention head.

reference computation (fp32):
    q = query @ Wq + bq ; k = key @ Wk + bk ; v = value @ Wv + bv
    out = softmax((q @ k^T) / 8) @ v

Sharding: 8 cores, core c -> (batch b = c//2, query-half h = c%2).
Each core computes attention for its 2048 query rows against the full 4096
keys/values of its batch.

Design (bf16 pipeline, host-transposed inputs; rel err ~7e-3 vs 2e-2 gate):
  - host supplies X^T slices in bf16 ([512, rows], c-major) so activations
    DMA straight into SBUF in the projection-ready layout: no PE transposes
    of X, no PSUM->SBUF staging copies, and half the HBM traffic of fp32.
    All weights are packed into one bf16 DMA, biases into one fp32 DMA.
  - projections on PE (bf16): lhsT = W [c-chunk, d], rhs = X^T chunk;
    bias folded into the mandatory PSUM->SBUF copy (DVE tensor_scalar_add).
    Qp^T [64, 2048] is duplicated to partitions 64:128 (sync-queue SBUF
    DMA); Kp^T is stored dual-half (even j-chunks on partitions 0:64, odd
    on 64:128) by issuing separate even/odd matmuls whose outputs land at
    partition offsets 0/64 via tile_position - no partition-shift DMA.
  - V is projected then PE-transposed to natural [rows, 66]; col 64 is
    all-ones (host pads Wv/bv) so the PV matmul also produces the softmax
    denominator; col 65 is zero padding.
  - scores^T tiles: lhsT = Kp^T[half, j-chunk] [64,128], rhs = Qp^T
    [64, 1024-i-slice] -> S^T [128 j, i] in PSUM; the two K=64 matmuls of a
    j-chunk pair occupy PE row-groups 0:64 / 64:128 (tile_position row
    tiling); exp fused with the 1/8 scale on ScalarE, output bf16 (no
    max-subtraction: |scores/8| <= ~3 so fp32 exp is safe).
  - PV: lhsT = v[j-chunk] [128, 66] bf16, rhs = P^T [128, i] bf16,
    accumulated over j in PSUM -> out^T [66, i] (row 64 = denominator).
  - epilogue: PE-transpose out^T, reciprocal on DVE, scale via Copy
    activation; the final epilogue runs its copies/scales on the (by then
    idle) ScalarE so the DVE chain is off the tail's critical path.
  - scheduling: phase A (prework + i-half 0) is PE-bound while phase B
    (i-half 1) is ACT-bound, so the PV of several ih0 pairs is deferred
    into phase B: their exp outputs stay live in the pt pool and drain
    while ih1 scores/exp keep ACT saturated through the transition. The
    first pair's exps are split per 512-column slice so ACT starts as soon
    as the first q block lands.
"""

import sys

if "/opt/trn_rl_repo" not in sys.path:
    sys.path.insert(0, "/opt/trn_rl_repo")

from contextlib import ExitStack

import numpy as np
import ml_dtypes

import concourse.bass as bass
import concourse.tile as tile
from concourse import bacc, mybir
from concourse.bass_utils import run_bass_kernel_spmd
from concourse.masks import make_identity

F32 = mybir.dt.float32
BF16 = mybir.dt.bfloat16
NP_BF16 = ml_dtypes.bfloat16

B, S, C, D = 4, 4096, 512, 64
D2 = D + 2          # v padded with [ones, zeros] cols
WW = 4 * (D + D + D2)  # packed weight row: 776 bf16 per partition
N_CORES = 8
SQ = S // 2          # query rows per core
NJ = S // 128        # 32 key chunks of 128 rows
NP_ = NJ // 2        # 16 chunk pairs
IH = SQ // 2         # 1024: i-half processed per PSUM residency
EXP = mybir.ActivationFunctionType.Exp
COPY = mybir.ActivationFunctionType.Copy

# ih0 pairs whose PV is deferred into phase B (spread so phase A's per-g
# PE load stays balanced against ACT)
DEFER = frozenset((3, 5, 7, 9, 11, 13, 15))

_CACHE = {}


def _emit(nc, tc, aps):
    qT_d, kT_d, vT_d, w_d, b_d, out_d = aps

    ctx = ExitStack()
    const = ctx.enter_context(tc.tile_pool(name="const", bufs=1))
    persist = ctx.enter_context(tc.tile_pool(name="persist", bufs=1))
    stage_p = ctx.enter_context(tc.tile_pool(name="stage", bufs=4))
    vt_p = ctx.enter_context(tc.tile_pool(name="vt", bufs=2))
    pt_p = ctx.enter_context(tc.tile_pool(name="pt", bufs=34))
    ep_p = ctx.enter_context(tc.tile_pool(name="ep", bufs=2))
    small_p = ctx.enter_context(tc.tile_pool(name="small", bufs=4))
    out_p = ctx.enter_context(tc.tile_pool(name="outp", bufs=2))
    # PSUM budget (8 banks): scratch 2x1 + st 2x2 + po 1x2 = 8
    pp_ps = ctx.enter_context(tc.tile_pool(name="ppps", bufs=2, space="PSUM"))
    st_ps = ctx.enter_context(tc.tile_pool(name="stps", bufs=2, space="PSUM"))
    po_ps = ctx.enter_context(tc.tile_pool(name="pops", bufs=1, space="PSUM"))

    # one packed DMA for all weights, one for all biases
    w_all = const.tile([128, 4, D + D + D2], BF16)
    nc.sync.dma_start(w_all[:].rearrange("p cc d -> p (cc d)"), w_d[:])
    wq_sb = w_all[:, :, 0:D]
    wk_sb = w_all[:, :, D : 2 * D]
    wvp_sb = w_all[:, :, 2 * D : 2 * D + D2]
    b_all = const.tile([128, 3], F32)
    nc.sync.dma_start(b_all[:], b_d[:])
    bq_sb = b_all[:D, 0:1]
    bk2_sb = b_all[:, 1:2]
    bvp_sb = b_all[:D2, 2:3]

    ident = const.tile([128, 128], BF16)

    qpt = persist.tile([128, SQ], BF16)      # Qp^T duplicated on both halves
    kpt = persist.tile([128, S // 2], BF16)  # Kp^T dual-half (even|odd chunks)
    v_sb = persist.tile([128, NJ, D2], BF16)  # v natural + ones col

    def load_block(x_d, g):
        """DMA 512 c x 512 rows of a host-transposed activation into SBUF.
        SWDGE (gpsimd): keeps engine HWDGE queues free; Pool is idle anyway."""
        stg = stage_p.tile([128, 4, 512], BF16, tag="stage")
        nc.gpsimd.dma_start(
            stg[:],
            x_d[:, g * 512 : (g + 1) * 512].rearrange("(cc p) r -> p cc r", p=128),
        )
        return stg

    def proj_q(g):
        stg = load_block(qT_d, g)
        pp = pp_ps.tile([128, 512], F32, tag="pp")
        for cc in range(4):
            nc.tensor.matmul(
                pp[:D, :], wq_sb[:, cc, :], stg[:, cc, :],
                start=(cc == 0), stop=(cc == 3),
            )
        sl = slice(g * 512, (g + 1) * 512)
        nc.vector.tensor_scalar_add(qpt[:D, sl], pp[:D, :], bq_sb)
        # duplicate to partitions 64:128 via the sync HWDGE queue (idle
        # mid-kernel, so its dep-wait blocks nothing)
        nc.sync.dma_start(qpt[D:, sl], qpt[:D, sl])

    def proj_k(g):
        # block g covers j-chunks 4g..4g+3; even chunks project to output
        # partitions 0:64, odd to 64:128 (tile_position col offset), so the
        # bias-add writes kpt's dual-half layout directly.
        stg = load_block(kT_d, g)
        pp = pp_ps.tile([128, 512], F32, tag="pp")
        for half in range(2):
            for cc in range(4):
                rhs = stg[:, cc, :].rearrange("p (c n) -> p c n", n=128)[:, half::2, :]
                nc.tensor.matmul(
                    pp[half * D : (half + 1) * D, :256],
                    wk_sb[:, cc, :],
                    rhs,
                    start=(cc == 0), stop=(cc == 3),
                    tile_position=(0, half * D),
                )
        sl = slice(g * 256, (g + 1) * 256)
        nc.vector.tensor_scalar_add(kpt[:D, sl], pp[:D, :256], bk2_sb[:D, :])
        nc.vector.tensor_scalar_add(kpt[D:, sl], pp[D:, :256], bk2_sb[D:, :])

    def proj_v(g):
        stg = load_block(vT_d, g)
        pp = pp_ps.tile([128, 512], F32, tag="pp")
        for cc in range(4):
            nc.tensor.matmul(
                pp[:D2, :], wvp_sb[:, cc, :], stg[:, cc, :],
                start=(cc == 0), stop=(cc == 3),
            )
        vt = vt_p.tile([D2, 512], BF16, tag="vt")
        nc.vector.tensor_scalar_add(vt[:], pp[:D2, :], bvp_sb)
        for r in range(4):
            vnp = pp_ps.tile([128, D2], BF16, tag="pp")
            nc.tensor.transpose(
                vnp[:], vt[:, r * 128 : (r + 1) * 128], ident[:D2, :D2]
            )
            nc.vector.tensor_copy(v_sb[:, g * 4 + r, :], vnp[:])

    def scores_exp(p, ih, nw=IH):
        # chunk pair p = chunks (2p, 2p+1): even on kpt rows 0:64, odd 64:128.
        # nw < 512 splits the matmul/exp into narrower column slices so the
        # first pair can start before the full q width has landed.
        sts = []
        for half in range(2):
            st = st_ps.tile([128, IH], F32, tag="st")
            pt = pt_p.tile([128, IH], BF16, tag="pt")
            for n in range(IH // 512):   # matmul out must fit one PSUM bank
                nc.tensor.matmul(
                    st[:, n * 512 : (n + 1) * 512],
                    kpt[half * D : (half + 1) * D, p * 128 : (p + 1) * 128],
                    qpt[half * D : (half + 1) * D,
                        ih * IH + n * 512 : ih * IH + (n + 1) * 512],
                    tile_position=(half * D, 0),
                )
                for m in range(512 // nw):
                    if nw < IH:
                        lo = n * 512 + m * nw
                        nc.scalar.activation(
                            pt[:, lo : lo + nw], st[:, lo : lo + nw],
                            EXP, scale=0.125,
                        )
            if nw == IH:
                nc.scalar.activation(pt[:], st[:], EXP, scale=0.125)
            sts.append(pt)
        return sts

    def pv(p, po, sts, first, last):
        # n-major on the last pair so po's low columns finish first and the
        # epilogue's first chunks unblock while the rest still accumulates
        order = [(h, n) for n in range(IH // 512) for h in range(2)] if last \
            else [(h, n) for h in range(2) for n in range(IH // 512)]
        for half, n in order:
            nc.tensor.matmul(
                po[:, n * 512 : (n + 1) * 512],
                v_sb[:, 2 * p + half, :],
                sts[half][:, n * 512 : (n + 1) * 512],
                start=(first and half == 0), stop=(last and half == 1),
            )

    def epilogue(ih, po, on_act):
        # chunked so po frees early and the final DMA launches with minimal
        # serial tail. on_act (the final epilogue, tail-latency critical):
        # chunks 0,1 run their copy/scale chain on DVE while 2,3 run on the
        # by-then-idle ACT, halving the serial tail.
        osb = out_p.tile([128, IH // 128, D], F32, tag="osb")
        for c in range(4):
            act_c = on_act and c >= 2
            ot = ep_p.tile([D2, 256], BF16, tag="ot")
            if act_c:
                nc.scalar.activation(ot[:], po[:, c * 256 : (c + 1) * 256], COPY)
            else:
                nc.vector.tensor_copy(ot[:], po[:, c * 256 : (c + 1) * 256])
            for tt in range(2):
                t = c * 2 + tt
                onat = pp_ps.tile([128, D2], BF16, tag="pp")
                nc.tensor.transpose(
                    onat[:], ot[:, tt * 128 : (tt + 1) * 128], ident[:D2, :D2]
                )
                rs = small_p.tile([128, 1], F32, tag="rs")
                nc.vector.reciprocal(rs[:], onat[:, D : D + 1])
                if act_c:
                    nc.scalar.activation(osb[:, t, :], onat[:, :D], COPY, scale=rs[:])
                else:
                    nc.vector.tensor_scalar_mul(osb[:, t, :], onat[:, :D], rs[:])
            if c % 2 == 1:
                nc.sync.dma_start(
                    out_d[ih * IH + (c - 1) * 256 : ih * IH + (c + 1) * 256, :]
                    .rearrange("(t p) d -> p t d", p=128),
                    osb[:, c * 2 - 2 : c * 2 + 2, :],
                )

    # emission order = per-engine program order; sequence so the first exp
    # lands as early as possible and ACT never waits on deferrable PE work
    po0 = po_ps.tile([D2, IH], F32, tag="po")
    deferred = []
    proj_k(0)
    proj_q(0)
    proj_q(1)
    ident32 = small_p.tile([128, 128], F32, tag="id32")
    make_identity(nc, ident32[:])           # after the head loads: Pool must
    nc.vector.tensor_copy(ident[:], ident32[:])  # not delay k0/q0 desc-gen
    for g in range(8):
        sts0 = scores_exp(2 * g, 0, nw=(512 if g == 0 else IH))
        if g < 7:
            proj_k(g + 1)       # keep the k pipeline one block ahead
        proj_v(g)
        if 2 * g in DEFER:
            deferred.append((2 * g, sts0))
        else:
            pv(2 * g, po0, sts0, first=(g == 0), last=False)
        sts1 = scores_exp(2 * g + 1, 0)
        if g in (5, 6):
            proj_q(g - 3)       # q cols 1024:2048, needed only for i-half 1
        if 2 * g + 1 in DEFER:
            deferred.append((2 * g + 1, sts1))
        else:
            pv(2 * g + 1, po0, sts1, first=False, last=False)

    # drain deferred ih0 PVs while ih1 scores/exp keep ACT saturated
    exped = {}
    for i, (p0, sts) in enumerate(deferred):
        exped[i] = scores_exp(i, 1)
        pv(p0, po0, sts, first=False, last=(i == len(deferred) - 1))
    epilogue(0, po0, on_act=False)
    # po1 phase: lookahead pipeline — scores/exp stay ~AH pairs ahead of PV
    AH = len(deferred)
    po1 = po_ps.tile([D2, IH], F32, tag="po")
    for p in range(NP_):
        if p + AH < NP_:
            exped[p + AH] = scores_exp(p + AH, 1)
        pv(p, po1, exped.pop(p), first=(p == 0), last=(p == NP_ - 1))
    epilogue(1, po1, on_act=True)
    ctx.close()


def _build(reps=1):
    nc = bacc.Bacc("TRN2", target_bir_lowering=False, debug=False, num_devices=N_CORES)
    aps = (
        nc.dram_tensor("qT", [C, SQ], BF16, kind="ExternalInput").ap(),
        nc.dram_tensor("kT", [C, S], BF16, kind="ExternalInput").ap(),
        nc.dram_tensor("vT", [C, S], BF16, kind="ExternalInput").ap(),
        nc.dram_tensor("w", [128, WW], BF16, kind="ExternalInput").ap(),
        nc.dram_tensor("b", [128, 3], F32, kind="ExternalInput").ap(),
        nc.dram_tensor("out", [SQ, D], F32, kind="ExternalOutput").ap(),
    )
    with tile.TileContext(nc) as tc:
        for _ in range(reps):
            _emit(nc, tc, aps)
    nc.compile()
    return nc


def get_nc():
    if "nc" not in _CACHE:
        _CACHE["nc"] = _build()
    return _CACHE["nc"]


def make_in_maps(query, key_, value, Wq, bq, Wk, bk, Wv, bv):
    query, key_, value, Wq, bq, Wk, bk, Wv, bv = (
        np.asarray(a, dtype=np.float32)
        for a in (query, key_, value, Wq, bq, Wk, bk, Wv, bv)
    )
    wvp = np.concatenate([Wv, np.zeros((C, 2), np.float32)], axis=1)
    # packed weights: per partition p, 4 c-chunks of [wq | wk | wvp] rows
    wcat = np.concatenate([Wq, Wk, wvp], axis=1)          # [512, 194]
    w = np.ascontiguousarray(
        wcat.reshape(4, 128, D + D + D2).transpose(1, 0, 2).reshape(128, WW)
        .astype(NP_BF16)
    )
    bvp = np.concatenate([bv, [1.0, 0.0], np.zeros(128 - D2, np.float32)])
    b = np.ascontiguousarray(
        np.stack(
            [
                np.concatenate([bq, np.zeros(64, np.float32)]),
                np.concatenate([bk, bk]),
                bvp,
            ],
            axis=1,
        ).astype(np.float32)
    )
    kT = [np.ascontiguousarray(key_[b_].T.astype(NP_BF16)) for b_ in range(B)]
    vT = [np.ascontiguousarray(value[b_].T.astype(NP_BF16)) for b_ in range(B)]
    in_maps = []
    for c in range(N_CORES):
        b_, h = divmod(c, 2)
        in_maps.append(
            {
                "qT": np.ascontiguousarray(
                    query[b_, h * SQ : (h + 1) * SQ, :].T.astype(NP_BF16)
                ),
                "kT": kT[b_],
                "vT": vT[b_],
                "w": w,
                "b": b,
            }
        )
    return in_maps


def assemble(results):
    out = np.empty((B, S, D), np.float32)
    for c in range(N_CORES):
        b_, h = divmod(c, 2)
        out[b_, h * SQ : (h + 1) * SQ, :] = results[c]["out"]
    return out


def kernel(query=None, key_=None, value=None, Wq=None, bq=None, Wk=None,
           bk=None, Wv=None, bv=None, key=None, **_):
    if key_ is None:
        key_ = key          # spec names this input "key"; reference uses "key_"
    nc = get_nc()
    in_maps = make_in_maps(query, key_, value, Wq, bq, Wk, bk, Wv, bv)
    res = run_bass_kernel_spmd(nc, in_maps, list(range(N_CORES)))
    return assemble(res.results)
